# revision 1
# baseline (speedup 1.0000x reference)
"""Causal single-head attention (B=2, S=4096, D=1024) with RoPE on 8 TRN2 NeuronCores.

Sharding: per batch element, the 32 kv chunks (128 rows) are dealt round-robin
to 4 cores (chunk k -> core k%4). Every core runs an identical 32-slot program:
slot j computes partial causal attention of query chunk j (128 rows) against
the first sched[j] = 128*(j//4+1) rows of the core's gathered kv buffer, with
host-provided additive causal masks (which also mask not-owned columns).
Cores return unnormalized partials (o_un, rowmax, rowsum); the host merges the
4 partial softmaxes per query row and normalizes.

All matmuls run in bf16 with fp32 PSUM accumulation. Q/K output features are
permuted (evens-then-odds) on the host so RoPE operates on contiguous halves;
the permutation cancels in Q.K^T. x arrives host-transposed and tile-blocked
so no on-device transposes of x are needed.
"""

import os
import sys

sys.path.insert(0, "/opt/trn_rl_repo")

import math
from contextlib import ExitStack

import ml_dtypes
import numpy as np

import concourse.bass as bass
import concourse.tile as tile
from concourse import bacc, mybir
from concourse.bass_utils import run_bass_kernel_spmd
from concourse.masks import make_identity

BF16 = mybir.dt.bfloat16
F32 = mybir.dt.float32
NPBF16 = ml_dtypes.bfloat16

B, S, D = 2, 4096, 1024
H = D // 2
C = 128                      # chunk rows
NQC = S // C                 # 32 query-chunk slots
NKVC = NQC // 4              # 8 kv chunks per core
NKV = NKVC * C               # 1024 resident kv rows per core
SCHED = [C * (j // 4 + 1) for j in range(NQC)]   # static kv window per slot
MOFF = [sum(SCHED[:j]) for j in range(NQC)]      # mask column offsets
MTOT = sum(SCHED)
QG = 512                     # phase-B query group rows (4 slots)
NG = S // QG                 # 8 groups
SPG = QG // C                # slots per group
SCALE = 1.0 / math.sqrt(D)
NEG = -30000.0

_CACHE = {}
KPHASE = os.environ.get("KPHASE", "all")


def _build():
    """Build + schedule the (core-uniform) Bass program once."""
    nc = bacc.Bacc("TRN2", target_bir_lowering=False, debug=False,
                   enable_asserts=False, num_devices=8)

    # host-blocked transposed x: xq_b[g, p, dc, s] = x[g*QG+s, dc*128+p]
    xq_b = nc.dram_tensor("xq_b", [NG, C, 8, QG], BF16, kind="ExternalInput").ap()
    # xkv_b[g, p, dc, s] = x[kvrows[g*128+s], dc*128+p]
    xkv_b = nc.dram_tensor("xkv_b", [NKVC, C, 8, C], BF16, kind="ExternalInput").ap()
    wqT = nc.dram_tensor("wqT", [D, D], BF16, kind="ExternalInput").ap()
    wkT = nc.dram_tensor("wkT", [D, D], BF16, kind="ExternalInput").ap()
    wvT = nc.dram_tensor("wvT", [D, D], BF16, kind="ExternalInput").ap()
    # cosq_b[g, p, c, s] = cos[g*QG+s, c*128+p]   (transposed rope tables)
    cosq_b = nc.dram_tensor("cosq_b", [NG, C, 4, QG], BF16, kind="ExternalInput").ap()
    sinq_b = nc.dram_tensor("sinq_b", [NG, C, 4, QG], BF16, kind="ExternalInput").ap()
    # natural rope tables for the gathered kv rows
    cos_kv = nc.dram_tensor("cos_kv", [NKV, H], BF16, kind="ExternalInput").ap()
    sin_kv = nc.dram_tensor("sin_kv", [NKV, H], BF16, kind="ExternalInput").ap()
    masks = nc.dram_tensor("masks", [C, MTOT], F32, kind="ExternalInput").ap()

    o_un = nc.dram_tensor("o_un", [NQC, C, D], F32, kind="ExternalOutput").ap()
    stats = nc.dram_tensor("stats", [C, NQC, 2], F32, kind="ExternalOutput").ap()

    with tile.TileContext(nc) as tc, ExitStack() as ctx:
        const_p = ctx.enter_context(tc.tile_pool(name="const", bufs=1))
        w_p = ctx.enter_context(tc.tile_pool(name="weights", bufs=1))
        kvres_p = ctx.enter_context(tc.tile_pool(name="kvres", bufs=1))
        stats_p = ctx.enter_context(tc.tile_pool(name="stats", bufs=1))

        ident = const_p.tile([C, C], BF16)
        make_identity(nc, ident[:])

        wq_sb = w_p.tile([C, 8, D], BF16, tag="wq")
        wk_sb = w_p.tile([C, 8, D], BF16, tag="wk")
        wv_sb = w_p.tile([C, 8, D], BF16, tag="wv")
        nc.sync.dma_start(wq_sb[:], wqT.rearrange("(dc p) e -> p dc e", p=C))
        nc.sync.dma_start(wk_sb[:], wkT.rearrange("(dc p) e -> p dc e", p=C))
        nc.sync.dma_start(wv_sb[:], wvT.rearrange("(dc p) e -> p dc e", p=C))

        kt_sb = kvres_p.tile([C, 8, NKV], BF16, tag="kt")     # [p, dc, kvpos]
        v_sb = kvres_p.tile([C, NKVC, D], BF16, tag="v")      # [p, kvchunk, d]
        stats_sb = stats_p.tile([C, NQC, 2], F32, tag="st")

        # Unified PSUM pools shared by all phases:
        #   mm512: [C,512] f32 slots (QT + S psums)          2 banks
        #   acc  : [C,1024] f32 slots (K, V, out psums)      4 banks
        #   tp   : [C,1024] bf16 slots (KT + PT transposes)  2 banks
        with tc.tile_pool(name="a1", bufs=2) as a1_p, \
             tc.tile_pool(name="b", bufs=2) as b_p, \
             tc.tile_pool(name="bq", bufs=2) as bq_p, \
             tc.tile_pool(name="bs", bufs=2) as bs_p, \
             tc.tile_pool(name="mmps", bufs=2, space="PSUM") as mmps_p, \
             tc.tile_pool(name="accps", bufs=2, space="PSUM") as accps_p, \
             tc.tile_pool(name="tpps", bufs=2, space="PSUM") as tpps_p:

            def emit_a1_chunk(g):
                rows = slice(g * C, (g + 1) * C)
                xt_sb = a1_p.tile([C, 8, C], BF16, tag="xtkv", name=f"xtkv_{g}")
                nc.sync.dma_start(xt_sb[:], xkv_b[g])
                ckv_sb = a1_p.tile([C, H], BF16, tag="ckv", name=f"ckv_{g}")
                skv_sb = a1_p.tile([C, H], BF16, tag="skv", name=f"skv_{g}")
                nc.sync.dma_start(ckv_sb[:], cos_kv[rows, :])
                nc.sync.dma_start(skv_sb[:], sin_kv[rows, :])

                k_ps = accps_p.tile([C, D], F32, tag="acc", name=f"kps_{g}")
                v_ps = accps_p.tile([C, D], F32, tag="acc", name=f"vps_{g}")
                for h in range(2):
                    cols = slice(h * 512, (h + 1) * 512)
                    for dc in range(8):
                        nc.tensor.matmul(k_ps[:, cols], xt_sb[:, dc, :],
                                         wk_sb[:, dc, cols],
                                         start=(dc == 0), stop=(dc == 7))
                    for dc in range(8):
                        nc.tensor.matmul(v_ps[:, cols], xt_sb[:, dc, :],
                                         wv_sb[:, dc, cols],
                                         start=(dc == 0), stop=(dc == 7))
                nc.scalar.copy(v_sb[:, g, :], v_ps[:])

                # rope K in natural layout (halves are real|imag after permutation)
                kr_sb = a1_p.tile([C, D], BF16, tag="kr", name=f"kr_{g}")
                t0 = a1_p.tile([C, H], BF16, tag="t0", name=f"kt0_{g}")
                t1 = a1_p.tile([C, H], BF16, tag="t1", name=f"kt1_{g}")
                re, im = k_ps[:, 0:H], k_ps[:, H:D]
                nc.vector.tensor_mul(t0[:], re, ckv_sb[:])
                nc.vector.tensor_mul(t1[:], im, skv_sb[:])
                nc.vector.tensor_sub(kr_sb[:, 0:H], t0[:], t1[:])
                nc.vector.tensor_mul(t0[:], re, skv_sb[:])
                nc.vector.tensor_mul(t1[:], im, ckv_sb[:])
                nc.vector.tensor_add(kr_sb[:, H:D], t0[:], t1[:])

                for dc in range(8):
                    tp = tpps_p.tile([C, 1024], BF16, tag="tp", name=f"ktp_{g}_{dc}")
                    nc.tensor.transpose(tp[:, 0:C], kr_sb[:, dc * C:(dc + 1) * C], ident[:])
                    nc.scalar.copy(kt_sb[:, dc, g * C:(g + 1) * C], tp[:, 0:C])

            def emit_b_group(g):
                xt_sb = b_p.tile([C, 8, QG], BF16, tag="xtq", name=f"xtq_{g}")
                nc.sync.dma_start(xt_sb[:], xq_b[g])
                ct_sb = b_p.tile([C, 4, QG], BF16, tag="ct", name=f"ct_{g}")
                st_sb = b_p.tile([C, 4, QG], BF16, tag="st", name=f"st_{g}")
                nc.sync.dma_start(ct_sb[:], cosq_b[g])
                nc.sync.dma_start(st_sb[:], sinq_b[g])

                qraw_sb = bq_p.tile([C, 8, QG], BF16, tag="qraw", name=f"qraw_{g}")
                for e in range(8):
                    qp = mmps_p.tile([C, 512], F32, tag="mm", name=f"qp_{g}_{e}")
                    for dc in range(8):
                        nc.tensor.matmul(qp[:, 0:QG], wq_sb[:, dc, e * C:(e + 1) * C],
                                         xt_sb[:, dc, :],
                                         start=(dc == 0), stop=(dc == 7))
                    nc.scalar.copy(qraw_sb[:, e, :], qp[:, 0:QG])

                qt_sb = bq_p.tile([C, 8, QG], BF16, tag="qt", name=f"qt_{g}")
                for ec in range(4):
                    cc, ss = ct_sb[:, ec, :], st_sb[:, ec, :]
                    re, im = qraw_sb[:, ec, :], qraw_sb[:, ec + 4, :]
                    t0 = b_p.tile([C, QG], BF16, tag="rt0", name=f"rt0_{g}_{ec}")
                    t1 = b_p.tile([C, QG], BF16, tag="rt1", name=f"rt1_{g}_{ec}")
                    nc.vector.tensor_mul(t0[:], re, cc)
                    nc.vector.tensor_mul(t1[:], im, ss)
                    nc.vector.tensor_sub(qt_sb[:, ec, :], t0[:], t1[:])
                    t2 = b_p.tile([C, QG], BF16, tag="rt2", name=f"rt2_{g}_{ec}")
                    t3 = b_p.tile([C, QG], BF16, tag="rt3", name=f"rt3_{g}_{ec}")
                    nc.vector.tensor_mul(t2[:], re, ss)
                    nc.vector.tensor_mul(t3[:], im, cc)
                    nc.vector.tensor_add(qt_sb[:, ec + 4, :], t2[:], t3[:])

                for jj in range(0 if KPHASE in ("a1", "qt") else SPG):
                    j = SPG * g + jj
                    W = SCHED[j]
                    qc = slice(jj * C, (jj + 1) * C)

                    m_sb = bs_p.tile([C, 1024], F32, tag="mask", name=f"m_{j}")
                    nc.sync.dma_start(m_sb[:, 0:W], masks[:, MOFF[j]:MOFF[j] + W])
                    sc_sb = bs_p.tile([C, 1024], F32, tag="scores", name=f"sc_{j}")
                    rmax = bs_p.tile([C, 1], F32, tag="rmax", name=f"rmax_{j}")

                    ntile = (W + 511) // 512
                    for t in range(ntile):
                        wt = min(512, W - t * 512)
                        cols = slice(t * 512, t * 512 + wt)
                        s_ps = mmps_p.tile([C, 512], F32, tag="mm", name=f"sps_{j}_{t}")
                        for dc in range(8):
                            nc.tensor.matmul(s_ps[:, 0:wt], qt_sb[:, dc, qc],
                                             kt_sb[:, dc, cols],
                                             start=(dc == 0), stop=(dc == 7))
                        nc.vector.tensor_add(sc_sb[:, cols], s_ps[:, 0:wt], m_sb[:, cols])

                    if KPHASE == "s":
                        return
                    nc.vector.tensor_reduce(rmax[:], sc_sb[:, 0:W],
                                            axis=mybir.AxisListType.X,
                                            op=mybir.AluOpType.max)
                    negm = bs_p.tile([C, 1], F32, tag="negm", name=f"negm_{j}")
                    nc.scalar.mul(negm[:], rmax[:], -SCALE)
                    p_sb = bs_p.tile([C, 1024], BF16, tag="p", name=f"p_{j}")
                    lsum = bs_p.tile([C, 1], F32, tag="lsum", name=f"lsum_{j}")
                    nc.scalar.activation(p_sb[:, 0:W], sc_sb[:, 0:W],
                                         mybir.ActivationFunctionType.Exp,
                                         bias=negm[:], scale=SCALE,
                                         accum_out=lsum[:])
                    nc.scalar.copy(stats_sb[:, j, 0:1], negm[:])
                    nc.scalar.copy(stats_sb[:, j, 1:2], lsum[:])

                    if KPHASE == "exp":
                        return
                    o_ps = accps_p.tile([C, D], F32, tag="acc", name=f"ops_{j}")
                    nsub = W // C
                    for s0 in range(0, nsub, 2):
                        npair = min(2, nsub - s0)
                        ptp = tpps_p.tile([C, 1024], BF16, tag="tp", name=f"ptp_{j}_{s0}")
                        for u in range(npair):
                            nc.tensor.transpose(ptp[:, u * C:(u + 1) * C],
                                                p_sb[:, (s0 + u) * C:(s0 + u + 1) * C],
                                                ident[:])
                        pt_sb = b_p.tile([C, 2 * C], BF16, tag="pt", name=f"pt_{j}_{s0}")
                        nc.scalar.copy(pt_sb[:, 0:npair * C], ptp[:, 0:npair * C])
                        for u in range(npair):
                            sI = s0 + u
                            for h in range(2):
                                cols = slice(h * 512, (h + 1) * 512)
                                nc.tensor.matmul(o_ps[:, cols], pt_sb[:, u * C:(u + 1) * C],
                                                 v_sb[:, sI, cols],
                                                 start=(sI == 0), stop=(sI == nsub - 1))
                    ob_sb = bs_p.tile([C, D], F32, tag="ob", name=f"ob_{j}")
                    nc.scalar.copy(ob_sb[:], o_ps[:])
                    nc.sync.dma_start(o_un[j], ob_sb[:])

            # interleaved emission: B group g needs kv chunks <= g
            emit_a1_chunk(0)
            emit_a1_chunk(1)
            ngroups = NG if KPHASE != "a1" else 0
            for g in range(ngroups):
                emit_b_group(g)
                if g + 2 < NKVC:
                    emit_a1_chunk(g + 2)
            if KPHASE == "a1":
                for g in range(2, NKVC):
                    emit_a1_chunk(g)

        if KPHASE in ("exp", "all"):
            nc.sync.dma_start(stats, stats_sb[:])

    nc.compile()
    return nc


def _prep_inputs(x, w_q, w_k, w_v, freqs_cos, freqs_sin):
    """Host-side per-core input maps (numpy)."""
    perm = np.concatenate([np.arange(0, D, 2), np.arange(1, D, 2)])
    wqT = np.ascontiguousarray(w_q[perm, :].T.astype(NPBF16))
    wkT = np.ascontiguousarray(w_k[perm, :].T.astype(NPBF16))
    wvT = np.ascontiguousarray(w_v.T.astype(NPBF16))
    cosq_b = np.ascontiguousarray(
        freqs_cos.astype(NPBF16).reshape(NG, QG, 4, C).transpose(0, 3, 2, 1))
    sinq_b = np.ascontiguousarray(
        freqs_sin.astype(NPBF16).reshape(NG, QG, 4, C).transpose(0, 3, 2, 1))

    in_maps = []
    for core in range(8):
        b, i = divmod(core, 4)
        kcs = np.arange(i, NQC, 4)
        kvrows = (kcs[:, None] * C + np.arange(C)[None, :]).reshape(-1)
        xb = np.asarray(x[b]).astype(NPBF16)
        xq_b = np.ascontiguousarray(
            xb.reshape(NG, QG, 8, C).transpose(0, 3, 2, 1))
        xkv_b = np.ascontiguousarray(
            xb[kvrows].reshape(NKVC, C, 8, C).transpose(0, 3, 2, 1))
        m = np.zeros((C, MTOT), np.float32)
        for j in range(NQC):
            W = SCHED[j]
            qg = j * C + np.arange(C)
            kg = kvrows[:W]
            m[:, MOFF[j]:MOFF[j] + W] = np.where(kg[None, :] <= qg[:, None], 0.0, NEG)
        in_maps.append({
            "xq_b": xq_b, "xkv_b": xkv_b,
            "wqT": wqT, "wkT": wkT, "wvT": wvT,
            "cosq_b": cosq_b, "sinq_b": sinq_b,
            "cos_kv": np.ascontiguousarray(freqs_cos[kvrows].astype(NPBF16)),
            "sin_kv": np.ascontiguousarray(freqs_sin[kvrows].astype(NPBF16)),
            "masks": m,
        })
    return in_maps


def _merge(results):
    """Host softmax-merge of per-core partials -> [B,S,D] f32."""
    out = np.zeros((B, S, D), np.float64)
    for b in range(B):
        for j in range(NQC):
            parts = []
            for i in range(min(j + 1, 4)):
                r = results[4 * b + i]
                mrow = -r["stats"][:, j, 0].astype(np.float64)
                lrow = r["stats"][:, j, 1].astype(np.float64)
                orow = r["o_un"][j].astype(np.float64)
                parts.append((mrow, lrow, orow))
            M = np.max(np.stack([p[0] for p in parts]), axis=0)
            num = np.zeros((C, D), np.float64)
            den = np.zeros((C,), np.float64)
            for mrow, lrow, orow in parts:
                w = np.exp(mrow - M)
                num += w[:, None] * orow
                den += w * lrow
            out[b, j * C:(j + 1) * C] = num / den[:, None]
    return out.astype(np.float32)


def kernel(x, w_q, w_k, w_v, freqs_cos, freqs_sin, _want_results=False, _trace=False):
    if "nc" not in _CACHE:
        _CACHE["nc"] = _build()
    nc = _CACHE["nc"]
    in_maps = _prep_inputs(np.asarray(x, np.float32), np.asarray(w_q, np.float32),
                           np.asarray(w_k, np.float32), np.asarray(w_v, np.float32),
                           np.asarray(freqs_cos, np.float32),
                           np.asarray(freqs_sin, np.float32))
    kr = run_bass_kernel_spmd(nc, in_maps, core_ids=list(range(8)), trace=_trace)
    out = _merge(kr.results)
    if _want_results:
        return out, kr
    return out

